# revision 1
# baseline (speedup 1.0000x reference)
"""Trainium2 Bass kernel for nn_MultiHeadedSelfAttention_86388972192276 (v2.1).

Sharding: 8 cores = 2 batches x 4 head-groups (4 heads each). Fully data
parallel, no collectives.

v2 over the baseline (240us):
  - fp8 (e4m3) PE operands: projections and attention numerator use
    DoubleRow (2 k-tiles/matmul, ~2x); scores are K=64 co-issued fp8
    pairs on row strips 0/64 (baseline pattern).
  - exp split across TWO engines: ACT native Exp for 9/16 key tiles,
    DVE Schraudolph for 7/16: i8 = sat(round(ps*c + B_p)) bitcast to
    fp8e4 ~= exp(s+logm); int8 saturation maps masked (-30000) scores
    to 0x80 = fp8 -0.0.
  - steps = (pair, 512-q-chunk) x 8 with projections woven in (K+Q0 up
    front, V during step 0, Q chunks ahead of use); ACT/DVE start
    ~10us in instead of ~80us.
  - PSUM: scores pipeline 3 deep (6 banks) + 2 numerator accumulators
    (2 banks); projections borrow scores slots.
  - blend: pq*(1-w)+bv*w precomputed on host (pqm0); h*(w/l) via the
    l-row DRAM bounce. Blend stages are deferred into the next step's
    kt loop, DMAs spread over sync + gpsimd queues, final muls on
    GPSIMD.

Bias handling (exact, host-folded):
  bk: shifts every score of a query equally -> softmax-invariant, drop.
  bq: adds bq.k'_j/8 per KEY -> folded into logm_eff / b8_eff.
  bv: h_full/l = h/l + bv -> folded into pqm0 as + bv*w.

Scale folding: wq_eff=16Wq, wk_eff=16Wk -> scores PSUM = 2048*s_true;
  wv_eff=32Wv and ones-col=32 -> v_dev = 32v, cancels in h/l.
"""

import sys
import numpy as np

sys.path.insert(0, "/opt/trn_rl_repo")

B, SQ, SV = 2, 2048, 2048
DV, DQ, DK, DO, H = 1024, 1280, 1024, 1024, 16
DH = 64
NCORES = 8
HPC = 4
NEG_MASK = -30000.0

SQW = 16.0
SKW = 16.0
SVW = 32.0
SSC = SQW * SKW * 8.0        # PSUM score = SSC * s_true
A8 = 8.0 / np.log(2.0)       # 11.5416...
B8 = 7.0 * 8.0 - 0.344       # e4m3 exp bias<<3, schraudolph-tuned

# engine per key-tile for exp: 'A' = ACT, 'D' = DVE
ENG = ['A', 'D', 'A', 'D', 'A', 'A', 'A', 'D',
       'A', 'D', 'A', 'D', 'A', 'A', 'D', 'A']

_CACHE = {}


def _build_nc():
    import concourse.bass as bass
    import concourse.tile as tile
    import concourse.mybir as mybir
    from concourse import bacc
    from contextlib import ExitStack

    fp32 = mybir.dt.float32
    fp8 = mybir.dt.float8e4
    i8 = mybir.dt.int8
    AF = mybir.ActivationFunctionType
    ALU = mybir.AluOpType
    DR = mybir.MatmulPerfMode.DoubleRow

    nc = bacc.Bacc(None)

    pqT = nc.dram_tensor("pqT", [128, 10, SQ], fp8, kind="ExternalInput")
    pvkT = nc.dram_tensor("pvkT", [128, 8, SV], fp8, kind="ExternalInput")
    wq_d = nc.dram_tensor("wq", [128, 10, 256], fp8, kind="ExternalInput")
    wk_d = nc.dram_tensor("wk", [128, 8, 256], fp8, kind="ExternalInput")
    wv_d = nc.dram_tensor("wv", [128, 8, 256], fp8, kind="ExternalInput")
    logm_d = nc.dram_tensor("logm", [128, 16], fp32, kind="ExternalInput")
    b8_d = nc.dram_tensor("b8", [128, 16], fp32, kind="ExternalInput")
    wg_d = nc.dram_tensor("wg", [128, HPC, 4, 4], fp32, kind="ExternalInput")
    wgr_d = nc.dram_tensor("wgr", [HPC, 4, 512], fp32, kind="ExternalInput")
    pqm0_d = nc.dram_tensor("pqm0", [HPC * DH, SQ], fp32, kind="ExternalInput")
    outT = nc.dram_tensor("outT", [HPC * DH, SQ], fp32, kind="ExternalOutput")

    with tile.TileContext(nc) as tc, ExitStack() as ctx:
        const = ctx.enter_context(tc.tile_pool(name="const", bufs=1))
        persist = ctx.enter_context(tc.tile_pool(name="persist", bufs=1))
        wpool = ctx.enter_context(tc.tile_pool(name="wpool", bufs=1))
        stream = ctx.enter_context(tc.tile_pool(name="stream", bufs=2))
        qstream = ctx.enter_context(tc.tile_pool(name="qstream", bufs=2))
        epool = ctx.enter_context(tc.tile_pool(name="epool", bufs=4))
        rows = ctx.enter_context(tc.tile_pool(name="rows", bufs=2))
        bcast = ctx.enter_context(tc.tile_pool(name="bcast", bufs=4))
        blend = ctx.enter_context(tc.tile_pool(name="blend", bufs=4))
        dscr = ctx.enter_context(tc.tile_pool(name="dscr", bufs=4,
                                              space="DRAM"))
        # PSUM (8 banks): scores 3x2 + numerator 2x1
        scps = ctx.enter_context(tc.tile_pool(name="scps", bufs=3,
                                              space="PSUM"))
        hps_p = ctx.enter_context(tc.tile_pool(name="hps", bufs=1,
                                               space="PSUM"))


        # ---- persistent activations (fp8) ----
        qT2 = [persist.tile([128, SQ], fp8, tag=f"qT2_{p}", name=f"qT2_{p}")
               for p in range(2)]
        kT2 = [persist.tile([128, SV], fp8, tag=f"kT2_{p}", name=f"kT2_{p}")
               for p in range(2)]
        # v_all[kk, svt, ch, 68]; col 64 = 32-ones (memset once);
        # cols 65-67 pad the ch-block stride to 272 B (DR LDW needs
        # k-tile step % 16 == 0)
        v_all = persist.tile([128, 16, HPC, 68], fp8, tag="v_all")
        nc.gpsimd.memset(v_all[:, :, :, 64], float(SVW))

        # ---- weights + inputs (column-chunked pv for early K-proj) ----
        wk_sb = wpool.tile([128, 8, 256], fp8)
        # two halves so K-proj's first DR matmuls (planes 0-3) can
        # issue before the full weight tensor lands
        nc.sync.dma_start(wk_sb[:, 0:4, :], wk_d[:, 0:4, :])
        nc.sync.dma_start(wk_sb[:, 4:8, :], wk_d[:, 4:8, :])
        pv_cs = []
        for cv in range(2):
            pv_c = stream.tile([128, 8, 1024], fp8, tag="pv", name=f"pv{cv}")
            for j in range(2):
                eng = nc.scalar if cv == 0 else nc.gpsimd
                eng.dma_start(
                    pv_c[:, :, bass.ds(j * 512, 512)],
                    pvkT[:, :, bass.ds(cv * 1024 + j * 512, 512)])
            pv_cs.append(pv_c)
        wq_sb = wpool.tile([128, 10, 256], fp8)
        nc.sync.dma_start(wq_sb[:], wq_d[:])
        wv_sb = wpool.tile([128, 8, 256], fp8)
        nc.sync.dma_start(wv_sb[:], wv_d[:])
        # out := pq*(1-w) + bv*w; attention term accumulated into it later
        nc.gpsimd.dma_start(outT[:, :], pqm0_d[:, :])
        logm_sb = const.tile([128, 16], fp32)
        nc.sync.dma_start(logm_sb[:], logm_d[:])
        b8_sb = const.tile([128, 16], fp32)
        nc.sync.dma_start(b8_sb[:], b8_d[:])
        wg_sb = const.tile([128, HPC, 4, 4], fp32)
        nc.sync.dma_start(wg_sb[:], wg_d[:])
        # tail-path w rows, parked on partition 64 (the l-row's lane)
        wgr_sb = const.tile([65, HPC, 4, 512], fp32)
        nc.sync.dma_start(wgr_sb[64:65, :, :, :], wgr_d[None, :, :, :])


        # ---- projections (borrow "sc" PSUM slots) ----
        def kproj_chunk(cv, j):
            for pr in range(2):
                ps = scps.tile([128, 2, 512], fp32, tag="sc")
                for t in range(4):
                    nc.tensor.matmul(
                        ps[:, 0, :],
                        wk_sb[:, 2 * t : 2 * t + 2,
                              pr * 128 : pr * 128 + 128],
                        pv_cs[cv][:, 2 * t : 2 * t + 2,
                                  bass.ds(j * 512, 512)],
                        start=(t == 0), stop=(t == 3),
                        perf_mode=DR,
                    )
                nc.scalar.copy(
                    kT2[pr][:, bass.ds(cv * 1024 + j * 512, 512)],
                    ps[:, 0, :])

        pq_cs = {}

        def load_pq_chunk(c):
            pq_c = qstream.tile([128, 10, 512], fp8, tag="pq", name=f"pq{c}")
            nc.sync.dma_start(pq_c[:], pqT[:, :, bass.ds(c * 512, 512)])
            pq_cs[c] = pq_c

        def qproj_chunk(c):
            for pr in range(2):
                ps = scps.tile([128, 2, 512], fp32, tag="sc")
                for t in range(5):
                    nc.tensor.matmul(
                        ps[:, 0, :],
                        wq_sb[:, 2 * t : 2 * t + 2,
                              pr * 128 : pr * 128 + 128],
                        pq_cs[c][:, 2 * t : 2 * t + 2, :],
                        start=(t == 0), stop=(t == 4),
                        perf_mode=DR,
                    )
                nc.scalar.copy(qT2[pr][:, bass.ds(c * 512, 512)],
                               ps[:, 0, :])
            del pq_cs[c]

        def vproj_tile(svt):
            cv, sv = divmod(svt, 8)
            ps = scps.tile([128, 2, 512], fp32, tag="sc")
            for t in range(4):
                nc.tensor.matmul(
                    ps[:, 0, 0:256],
                    pv_cs[cv][:, 2 * t : 2 * t + 2,
                              bass.ds(sv * 128, 128)],
                    wv_sb[:, 2 * t : 2 * t + 2, :],
                    start=(t == 0), stop=(t == 3),
                    perf_mode=DR,
                )
            nc.scalar.copy(
                v_all[:, svt, :, 0:64],
                ps[:, 0, 0:256].rearrange("p (c f) -> p c f", c=4))

        kproj_chunk(0, 0)
        load_pq_chunk(0)
        kproj_chunk(0, 1)
        kproj_chunk(1, 0)
        kproj_chunk(1, 1)
        qproj_chunk(0)

        # ---- attention steps ----
        combos = [(pr, c) for c in range(4) for pr in range(2)]

        def emit_scores(pr, c, kt):
            ps = scps.tile([128, 2, 512], fp32, tag="sc")
            for hh in range(2):
                ro = 64 * hh
                nc.tensor.matmul(
                    ps[:, hh, :],
                    kT2[pr][ro : ro + 64, bass.ds(kt * 128, 128)],
                    qT2[pr][ro : ro + 64, bass.ds(c * 512, 512)],
                    start=True, stop=True,
                )
            return ps

        def emit_exp(ps, e_t, kt):
            tpl = kt & 1
            if ENG[kt] == 'A':
                nc.scalar.activation(
                    e_t[:, tpl, :, :], ps[:], AF.Exp,
                    bias=logm_sb[:, kt : kt + 1], scale=float(1.0 / SSC))
            else:
                nc.vector.tensor_scalar(
                    e_t[:, tpl, :, :].bitcast(i8), ps[:],
                    float(A8 / SSC), b8_sb[:, kt : kt + 1],
                    ALU.mult, ALU.add)

        def emit_numer(e_t, hps2, pr, dkt):
            for hh in range(2):
                nc.tensor.matmul(
                    hps2[hh][:],
                    v_all[:, 2 * dkt : 2 * dkt + 2, 2 * pr + hh, 0:65],
                    e_t[:, :, hh, :],
                    start=(dkt == 0), stop=(dkt == 7),
                    perf_mode=DR,
                )

        # ---- deferred blend: stages executed inside the NEXT step ----
        def blend_s0(st, hh, hps, dmae=None, act_hcp=False):
            # evacuate PSUM into the shared [65, 2, 512] tile; l bounce
            if "hcpP" not in st:
                st["hcpP"] = blend.tile([65, 2, 512], fp32, tag="hcpP",
                                        name="hcpP")
                st["m1bP"] = bcast.tile([64, 2, 512], fp32, tag="m1bP",
                                        name="m1bP")
            dmae = dmae or nc.sync
            if act_hcp:
                nc.scalar.copy(st["hcpP"][:, hh, :], hps[:])
            else:
                nc.vector.tensor_copy(st["hcpP"][:, hh, :], hps[:])
            ld = dscr.tile([1, 512], fp32, tag=f"ld{hh}", name=f"ld{hh}")
            dmae.dma_start(ld[:], st["hcpP"][64:65, hh, :])
            lz = rows.tile([128, 4], fp32, tag=f"lz{hh}", name=f"lz{hh}")
            dmae.dma_start(lz[:], ld.rearrange("o (p f) -> p (o f)", f=4))
            st[hh]["lz"] = lz

        def blend_s1(st, hh, dmae=None):
            # 1/l, *w (DVE, tiny), bounce out + broadcast into m1bP half
            s = st[hh]
            dmae = dmae or nc.sync
            rl = rows.tile([128, 4], fp32, tag=f"rl{hh}", name=f"rl{hh}")
            nc.vector.reciprocal(rl[:], s["lz"][:])
            m8 = rows.tile([128, 4], fp32, tag=f"m8{hh}", name=f"m8{hh}")
            nc.vector.tensor_tensor(
                m8[:], wg_sb[:, s["ch"], s["c"], :], rl[:], ALU.mult)
            md = dscr.tile([1, 512], fp32, tag=f"md{hh}", name=f"md{hh}")
            dmae.dma_start(md.rearrange("o (p f) -> p (o f)", f=4), m8[:])
            dmae.dma_start(st["m1bP"][:, hh, :],
                           md[0:1, :].to_broadcast((64, 512)))

        def blend_s2(st, pr, c, eng=None, final=False):
            # a = h*(w/l) for both heads in one op; out += a (accum DMA).
            # Final step: explicit add with a prefetched pqm0 tile and a
            # plain write, so the kernel doesn't end on an accum
            # read-modify-write and its ring drain.
            eng = eng or nc.gpsimd
            aP = blend.tile([64, 2, 512], fp32, tag="aP", name="aP")
            eng.tensor_tensor(
                aP[:], st["hcpP"][0:64, :, :], st["m1bP"][:], ALU.mult)
            dst = outT[bass.ds(2 * pr * 64, 128), bass.ds(c * 512, 512)]
            if final:
                oP = blend.tile([64, 2, 512], fp32, tag="oP", name="oP")
                eng.tensor_tensor(oP[:], aP[:], st["btP"][:], ALU.add)
                nc.sync.dma_start(
                    dst.rearrange("(hh d) q -> d hh q", hh=2), oP[:])
            else:
                nc.gpsimd.dma_start(
                    dst.rearrange("(hh d) q -> d hh q", hh=2), aP[:],
                    accum_op=ALU.add)

        pending = {}   # blend state of the previous step
        prev_hps = None
        prev_prc = None

        for si, (pr, c) in enumerate(combos):
            st = {0: {"ch": 2 * pr, "c": c}, 1: {"ch": 2 * pr + 1, "c": c}}
            hps2 = [hps_p.tile([65, 512], fp32, tag=f"hT{hh}", name=f"hT{hh}")
                    for hh in range(2)]
            e_t = None
            for kt in range(16):
                ps = emit_scores(pr, c, kt)
                if kt & 1 == 0:
                    e_t = epool.tile([128, 2, 2, 512], fp8, tag="e",
                                     name=f"e{si}_{kt // 2}")
                emit_exp(ps, e_t, kt)
                if si == 0:
                    vproj_tile(kt)
                if kt & 1 == 1:
                    emit_numer(e_t, hps2, pr, kt // 2)
                # previous step's blend stages, spread through this step
                if prev_hps is not None:
                    if kt == 0:
                        blend_s0(pending, 0, prev_hps[0])
                    elif kt == 1:
                        blend_s0(pending, 1, prev_hps[1])
                    elif kt == 3:
                        blend_s1(pending, 0)
                    elif kt == 4:
                        blend_s1(pending, 1)
                    elif kt == 6:
                        blend_s2(pending, *prev_prc,
                                 final=(si == 7))
                # input/projection staging for later steps
                if si == 0 and kt == 2:
                    load_pq_chunk(1)
                if si == 1 and kt == 4:
                    qproj_chunk(1)
                if si == 1 and kt == 8:
                    load_pq_chunk(2)
                if si == 3 and kt == 4:
                    qproj_chunk(2)
                if si == 3 and kt == 8:
                    load_pq_chunk(3)
                if si == 5 and kt == 4:
                    qproj_chunk(3)
                if si >= 6 and kt == 2:
                    btP = blend.tile([64, 2, 512], fp32, tag="btP",
                                     name="btP")
                    src_bt = pqm0_d[bass.ds(2 * pr * 64, 128),
                                    bass.ds(c * 512, 512)]
                    nc.sync.dma_start(
                        btP[:], src_bt.rearrange("(hh d) q -> d hh q", hh=2))
                    st["btP"] = btP
            pending, prev_hps, prev_prc = st, hps2, (pr, c)

        # final step's blend (tail): 1/l = exp(-ln l) on idle ACT in
        # row form (lane 64), *w on DVE, one DRAM bounce for the
        # partition broadcast — 2 DMA hops instead of 4.
        fpr, fc = prev_prc
        st = pending
        st["hcpP"] = blend.tile([65, 2, 512], fp32, tag="hcpP",
                                name="hcpP")
        st["m1bP"] = bcast.tile([64, 2, 512], fp32, tag="m1bP",
                                name="m1bP")
        nc.scalar.copy(st["hcpP"][:, 0, :], prev_hps[0][:])
        nc.vector.tensor_copy(st["hcpP"][:, 1, :], prev_hps[1][:])
        t1 = rows.tile([65, 2, 512], fp32, tag="t1", name="t1")
        nc.scalar.activation(t1[64:65, :, :], st["hcpP"][64:65, :, :],
                             AF.Ln)
        r1 = rows.tile([65, 2, 512], fp32, tag="r1", name="r1")
        nc.scalar.activation(r1[64:65, :, :], t1[64:65, :, :], AF.Exp,
                             scale=-1.0)
        m1r = rows.tile([65, 2, 512], fp32, tag="m1r", name="m1r")
        nc.vector.tensor_tensor(
            m1r[64:65, :, :], r1[64:65, :, :],
            wgr_sb[64:65, 2 * fpr : 2 * fpr + 2, fc, :], ALU.mult)
        mdP = dscr.tile([1, 1024], fp32, tag="mdP", name="mdP")
        nc.sync.dma_start(
            mdP[:], m1r[64:65, :, :].rearrange("o a q -> o (a q)"))
        nc.sync.dma_start(
            st["m1bP"][:, :, :],
            mdP[0:1, :].rearrange("o (a q) -> o a q", a=2)
            .to_broadcast((64, 2, 512)))
        blend_s2(pending, fpr, fc, eng=nc.vector, final=True)

    nc.finalize()
    return nc


def _get_nc():
    if "nc" not in _CACHE:
        _CACHE["nc"] = _build_nc()
    return _CACHE["nc"]


def _prep_core_inputs(c, pre_value_key, pre_query, value_key_masks,
                      value_key_counts, Wq, bq, Wk, bk, Wv, bv,
                      overall_gain, overall_bias):
    import ml_dtypes
    f8 = ml_dtypes.float8_e4m3
    f = np.float32

    b = c // 4
    h0 = (c % 4) * HPC
    cols = slice(h0 * DH, h0 * DH + HPC * DH)

    pqT = np.ascontiguousarray(
        pre_query[b].T.reshape(10, 128, SQ).transpose(1, 0, 2))
    pvkT = np.ascontiguousarray(
        pre_value_key[b].T.reshape(8, 128, SV).transpose(1, 0, 2))

    wq = np.ascontiguousarray(
        (Wq[:, cols] * SQW).reshape(10, 128, 256).transpose(1, 0, 2))
    wk = np.ascontiguousarray(
        (Wk[:, cols] * SKW).reshape(8, 128, 256).transpose(1, 0, 2))
    wv = np.ascontiguousarray(
        (Wv[:, cols] * SVW).reshape(8, 128, 256).transpose(1, 0, 2))

    mask_b = value_key_masks[b]
    logm = np.where(mask_b == 0, np.float32(NEG_MASK), np.float32(0.0))
    # fold bq: per-key addend bq . k'_j / 8 (k' biasless)
    u = Wk[:, cols] @ bq[cols]
    kbq = (pre_value_key[b] @ u) / 8.0
    keyb = logm + kbq.astype(np.float32)
    logm_st = np.ascontiguousarray(keyb.reshape(16, 128).T.astype(f))
    b8m = B8 + A8 * keyb
    b8_st = np.ascontiguousarray(b8m.reshape(16, 128).T.astype(f))

    # ---- host gate (exact, generic) ----
    msum = np.float32(mask_b.sum())
    km256 = (mask_b @ pre_value_key[b]) @ (Wk[:, cols] / 8.0) \
        + (bk[cols] / 8.0) * msum
    gain = overall_gain.reshape(H)
    bias = overall_bias.reshape(H)
    cnt = np.float32(value_key_counts[b])
    km2 = km256.reshape(HPC, DH)
    U = np.einsum("dhk,hk->dh", Wq[:, cols].reshape(DQ, HPC, DH), km2)
    C = (bq[cols].reshape(HPC, DH) * km2).sum(1)
    pooled = pre_query[b] @ U + C
    z = pooled * (gain[h0 : h0 + HPC] / cnt) + bias[h0 : h0 + HPC]
    w = 1.0 / (1.0 + np.exp(-z.astype(np.float64)))
    w = w.astype(np.float32)  # [SQ, HPC]

    wg = np.ascontiguousarray(
        w.T.reshape(HPC, 4, 128, 4).transpose(2, 0, 1, 3))
    wgr = np.ascontiguousarray(w.T.reshape(HPC, 4, 512))
    pq_split = pre_query[b, :, cols].reshape(SQ, HPC, DH)
    bv_h = bv[cols].reshape(HPC, DH)
    pqm0 = pq_split * (1.0 - w)[:, :, None] + bv_h[None] * w[:, :, None]
    pqm0T = np.ascontiguousarray(pqm0.reshape(SQ, 256).T)

    return {
        "pqT": pqT.astype(f8),
        "pvkT": pvkT.astype(f8),
        "wq": np.clip(wq, -240, 240).astype(f8),
        "wk": np.clip(wk, -240, 240).astype(f8),
        "wv": np.clip(wv, -240, 240).astype(f8),
        "logm": logm_st,
        "b8": b8_st,
        "wg": wg.astype(f, copy=False),
        "wgr": wgr.astype(f, copy=False),
        "pqm0": pqm0T.astype(f, copy=False),
    }


def kernel(trace=False, **inputs):
    from concourse.bass_utils import run_bass_kernel_spmd

    inputs = {k: np.asarray(v, np.float32) for k, v in inputs.items()}
    nc = _get_nc()
    in_maps = [_prep_core_inputs(c, **inputs) for c in range(NCORES)]
    res = run_bass_kernel_spmd(nc, in_maps, core_ids=list(range(NCORES)),
                               trace=trace)
    _CACHE["last_result"] = res

    pre_query = inputs["pre_query"]
    out = np.empty((B, SQ, DQ), np.float32)
    out[:, :, DO:] = pre_query[:, :, DO:]
    for c in range(NCORES):
        b = c // 4
        h0 = (c % 4) * HPC
        oT = res.results[c]["outT"]
        out[b, :, h0 * DH : h0 * DH + HPC * DH] = oT.T
    return out

